# revision 1
# baseline (speedup 1.0000x reference)
"""CT projector 2D forward (nn_CTProjector2D) — Trainium2 Bass kernel.

Strategy (8 NeuronCores, data-parallel over rays; image replicated):
  - 16384 rays/core, processed as 128 tiles of 128 rays x 768 t-values
    (rays in partitions, t along the free dimension).
  - Per-segment pixel indices are computed with DVE ops that reproduce the
    reference's fp32 arithmetic bit-exactly (for the graded M=I, b=0 case):
    x_t = t*ddx + sx, rowf = 0.5*(x_s+x_{s+1}), round via the +1.5*2^23
    magic constant (RNE, same as jnp.round).
  - Out-of-bounds segments cost nothing: we gather from a zero-padded image
    with guard bands, so OOB pixels read 0 exactly like the reference's
    `where(valid, ...)` masking.
  - The gather: Trainium has no per-element gather engine; the fastest
    measured primitive is the SWDGE `dma_gather` (MoE token gather) at its
    256 B minimum granule.  Each segment fetches its 64-px aligned block of
    the padded image from DRAM (int16 block ids, 4 parallel SWDGE queues),
    and the pixel is then selected on DVE with an iota/is_equal mask +
    multiply + inner-axis reduce.  The ucode wants indices "wrapped in 16
    partitions and replicated"; that layout is built on-device via a DRAM
    round-trip with strided access patterns.
  - seg_len = (t_{s+1}-t_s)*|dst-src| (== the reference's sqrt form to
    ~1e-7), fused with the per-ray reduction in one
    scalar_tensor_tensor(accum_out=...) op.
"""

import os
import sys

for _p in ("/opt/trn_rl_repo", "/root/.axon_site/_ro/trn_rl_repo"):
    if os.path.isdir(_p) and _p not in sys.path:
        sys.path.insert(0, _p)

import numpy as np

import concourse.bacc as bacc
import concourse.mybir as mybir
import concourse.tile as tile
from concourse import bass, bass_utils, library_config

F32 = mybir.dt.float32
I16 = mybir.dt.int16
OP = mybir.AluOpType

N_CORES = 8
P = 128                 # rays per tile (partitions)
GR = 64                 # gather granule: 64 f32 = 256 B (dma_gather minimum)
CH = 64                 # segments per dma_gather chunk
NCHUNK = 12             # chunks per tile (12*64 = 768 slots, 767 real)
MAGIC = np.float32(1.5 * 2.0**23)

_PROGRAM_CACHE: dict = {}


def _build_program(n_tiles: int, n_int: int, pw: int, nblk: int,
                   shift_r: float, shift_c: float, n_queues: int = 4):
    nseg = n_int - 1          # 767
    nsegp = NCHUNK * CH       # 768 (padded segment slots)
    assert nsegp == n_int
    nc = bacc.Bacc("TRN2", target_bir_lowering=False, debug=False,
                   num_devices=N_CORES, num_swdge_queues=n_queues)

    t_d = nc.dram_tensor("t", [n_tiles * P, n_int], F32, kind="ExternalInput").ap()
    cf_d = nc.dram_tensor("cf", [P, 5 * n_tiles], F32, kind="ExternalInput").ap()
    img_d = nc.dram_tensor("pimg", [nblk, GR], F32, kind="ExternalInput").ap()
    io_d = nc.dram_tensor("iota64", [1, GR], F32, kind="ExternalInput").ap()
    out_d = nc.dram_tensor("out", [n_tiles * P], F32, kind="ExternalOutput").ap()

    t_v = t_d.rearrange("(a p) k -> a p k", p=P)
    out_v = out_d.rearrange("(a p) -> p a", p=P)

    with tile.TileContext(nc) as tc:
        with (
            tc.tile_pool(name="singles", bufs=1) as singles,
            tc.tile_pool(name="work", bufs=2) as work,
            tc.tile_pool(name="big", bufs=4) as big,
            tc.tile_pool(name="big2", bufs=2) as big2,
            tc.tile_pool(name="dram", bufs=2, space="DRAM") as dram,
        ):
            nc.gpsimd.load_library(library_config.mlp)
            cf = singles.tile([P, 5 * n_tiles], F32)
            nc.sync.dma_start(out=cf[:], in_=cf_d[:])
            sino = singles.tile([P, n_tiles], F32)
            iota64 = singles.tile([P, GR], F32)
            io_b = bass.AP(io_d.tensor, io_d.offset,
                           [[0, P], io_d.ap[1]])
            nc.sync.dma_start(out=iota64[:], in_=io_b)

            def cfs(j, a):
                k = j * n_tiles + a
                return cf[:, k:k + 1]

            inv64 = float(np.float32(1.0 / 64.0))
            for a in range(n_tiles):
                t_t = work.tile([P, n_int], F32, tag="t")
                nc.sync.dma_start(out=t_t[:], in_=t_v[a, :, :])

                # projected line coords (bit-exact with ref for M=I, b=0)
                x = work.tile([P, n_int], F32, tag="x")
                nc.vector.tensor_scalar(out=x[:], in0=t_t[:],
                                        scalar1=cfs(0, a), scalar2=cfs(1, a),
                                        op0=OP.mult, op1=OP.add)
                y = work.tile([P, n_int], F32, tag="y")
                nc.vector.tensor_scalar(out=y[:], in0=t_t[:],
                                        scalar1=cfs(2, a), scalar2=cfs(3, a),
                                        op0=OP.mult, op1=OP.add)
                # xs := row' = round(0.5*(x_s+x_{s+1})) + shift_r   (in-place)
                xs = work.tile([P, nsegp], F32, tag="xs")
                nc.vector.tensor_tensor(out=xs[:, 0:nseg], in0=x[:, 0:nseg],
                                        in1=x[:, 1:n_int], op=OP.add)
                nc.vector.memset(xs[:, nseg:nsegp], 0.0)
                nc.vector.tensor_scalar(out=xs[:], in0=xs[:],
                                        scalar1=0.5, scalar2=float(MAGIC),
                                        op0=OP.mult, op1=OP.add)
                nc.vector.tensor_scalar(out=xs[:], in0=xs[:],
                                        scalar1=float(MAGIC - np.float32(shift_r)),
                                        scalar2=None, op0=OP.subtract)
                ys = work.tile([P, nsegp], F32, tag="ys")
                nc.vector.tensor_tensor(out=ys[:, 0:nseg], in0=y[:, 0:nseg],
                                        in1=y[:, 1:n_int], op=OP.add)
                nc.vector.memset(ys[:, nseg:nsegp], 0.0)
                nc.vector.tensor_scalar(out=ys[:], in0=ys[:],
                                        scalar1=0.5, scalar2=float(MAGIC),
                                        op0=OP.mult, op1=OP.add)
                nc.vector.tensor_scalar(out=ys[:], in0=ys[:],
                                        scalar1=float(MAGIC - np.float32(shift_c)),
                                        scalar2=None, op0=OP.subtract)

                # flat = row'*PW + col'  (exact integer in f32)
                flat = work.tile([P, nsegp], F32, tag="flat")
                nc.vector.scalar_tensor_tensor(out=flat[:], in0=xs[:],
                                               scalar=float(pw), in1=ys[:],
                                               op0=OP.mult, op1=OP.add)
                # xs := blk = floor(flat/64)  (in-place reuse of xs)
                nc.vector.tensor_scalar(out=xs[:], in0=flat[:],
                                        scalar1=inv64, scalar2=-0.4921875,
                                        op0=OP.mult, op1=OP.add)
                nc.vector.tensor_scalar(out=xs[:], in0=xs[:],
                                        scalar1=float(MAGIC), scalar2=float(MAGIC),
                                        op0=OP.add, op1=OP.subtract)
                # m = flat - 64*blk
                m = work.tile([P, nsegp], F32, tag="m")
                nc.vector.scalar_tensor_tensor(out=m[:], in0=xs[:],
                                               scalar=-64.0, in1=flat[:],
                                               op0=OP.mult, op1=OP.add)
                blk16 = work.tile([P, nsegp], I16, tag="blk16")
                nc.vector.tensor_copy(blk16[:], xs[:])

                # dt
                dt = work.tile([P, nseg], F32, tag="dt")
                nc.vector.tensor_tensor(out=dt[:], in0=t_t[:, 1:n_int],
                                        in1=t_t[:, 0:nseg], op=OP.subtract)

                # ---- wrapped-index layout via DRAM round trip ----
                # wrapped[q, g, c*8+k] = blk16[16k+q, CH*g+c]
                shuf = dram.tile([16, NCHUNK, CH, 8], I16, tag="shuf")
                for k in range(8):
                    nc.sync.dma_start(
                        out=shuf[:, :, :, k],
                        in_=blk16[16 * k:16 * k + 16, :].rearrange(
                            "q (g c) -> q g c", g=NCHUNK))
                wrapped = work.tile([P, NCHUNK, CH * 8], I16, tag="wrapped")
                src = shuf[:].rearrange("q g c k -> q (g c k)")
                src_b = bass.AP(src.tensor, src.offset,
                                [[0, 8], src.ap[0], src.ap[1]])
                nc.sync.dma_start(
                    out=wrapped[:].rearrange("p g i -> p (g i)"),
                    in_=src_b)

                # ---- gather + select per chunk ----
                val = work.tile([P, nsegp], F32, tag="val")
                for g in range(NCHUNK):
                    g64 = big.tile([P, CH, GR], F32, tag="g64")
                    nc.gpsimd.dma_gather(
                        out_ap=g64[:], in_ap=img_d[:, :],
                        idxs_ap=wrapped[:, g, :],
                        num_idxs=CH * P, num_idxs_reg=CH * P,
                        elem_size=GR,
                        queue_num=(a * NCHUNK + g) % n_queues,
                        single_packet=False)
                    mask = big2.tile([P, CH, GR], F32, tag="mask")
                    i_ap = iota64[:]
                    iota_b = bass.AP(i_ap.tensor, i_ap.offset,
                                     [i_ap.ap[0], [0, CH], i_ap.ap[1]])
                    m_sl = m[:, g * CH:(g + 1) * CH]
                    m_b = bass.AP(m_sl.tensor, m_sl.offset,
                                  [m_sl.ap[0], m_sl.ap[1], [0, GR]])
                    nc.vector.tensor_tensor(out=mask[:], in0=iota_b, in1=m_b,
                                            op=OP.is_equal)
                    nc.vector.tensor_tensor(out=mask[:], in0=mask[:], in1=g64[:],
                                            op=OP.mult)
                    nc.vector.tensor_reduce(
                        out=val[:, g * CH:(g + 1) * CH],
                        in_=mask[:], op=OP.add, axis=mybir.AxisListType.X)

                # sino[:, a] = sum((val * L) * dt)
                scr = work.tile([P, nseg], F32, tag="scr")
                nc.vector.scalar_tensor_tensor(out=scr[:], in0=val[:, 0:nseg],
                                               scalar=cfs(4, a), in1=dt[:],
                                               op0=OP.mult, op1=OP.mult,
                                               accum_out=sino[:, a:a + 1])

            nc.sync.dma_start(out=out_v[:, :], in_=sino[:])

    nc.compile()
    return nc


def _prep(image, t_sorted, M, b, src, dst):
    """Host-side O(n_ray) prep: per-ray line coefficients + padded image."""
    f32 = np.float32
    image = np.ascontiguousarray(image, dtype=f32)
    t_sorted = np.ascontiguousarray(t_sorted, dtype=f32)
    M = np.asarray(M, dtype=f32)
    b = np.asarray(b, dtype=f32)
    src = np.asarray(src, dtype=f32)
    dst = np.asarray(dst, dtype=f32)

    n_row, n_col = image.shape
    n_ray, n_int = t_sorted.shape

    Minv = np.linalg.inv(M.astype(np.float64)).astype(f32)
    sx, sy = src[:, 0], src[:, 1]
    ddx = dst[:, 0] - sx
    ddy = dst[:, 1] - sy

    c1x = Minv[0, 0] * ddx + Minv[0, 1] * ddy
    c2x = (Minv[0, 0] * sx + Minv[0, 1] * sy) - (Minv[0, 0] * b[0] + Minv[0, 1] * b[1])
    c1y = Minv[1, 0] * ddx + Minv[1, 1] * ddy
    c2y = (Minv[1, 0] * sx + Minv[1, 1] * sy) - (Minv[1, 0] * b[0] + Minv[1, 1] * b[1])
    L = np.sqrt(ddx.astype(np.float64) ** 2 + ddy.astype(np.float64) ** 2).astype(f32)

    rlo = np.floor(min(np.min(c2x), np.min(c1x + c2x))) - 2.0
    rhi = np.ceil(max(np.max(c2x), np.max(c1x + c2x))) + 2.0
    clo = np.floor(min(np.min(c2y), np.min(c1y + c2y))) - 2.0
    chi = np.ceil(max(np.max(c2y), np.max(c1y + c2y))) + 2.0
    rlo, clo = min(rlo, 0.0), min(clo, 0.0)
    rhi, chi = max(rhi, float(n_row)), max(chi, float(n_col))
    shift_r = -rlo
    shift_c = -clo
    ph = int(rhi - rlo) + 2
    pw = int(chi - clo) + 2
    ph = -(-ph // 64) * 64
    pw = -(-pw // 64) * 64
    # block table must be int16-indexable
    assert ph * pw // GR < 32768, (ph, pw)

    pimg = np.zeros((ph, pw), dtype=f32)
    pimg[int(shift_r):int(shift_r) + n_row,
         int(shift_c):int(shift_c) + n_col] = image
    pimg = pimg.reshape(-1, GR)

    return (t_sorted, c1x, c2x, c1y, c2y, L, pimg, ph, pw, shift_r, shift_c,
            n_ray, n_int, n_col)


def _get_program(n_tiles, n_int, pw, nblk, shift_r, shift_c):
    key = (n_tiles, n_int, pw, nblk, float(shift_r), float(shift_c))
    if key not in _PROGRAM_CACHE:
        _PROGRAM_CACHE[key] = _build_program(n_tiles, n_int, pw, nblk,
                                             shift_r, shift_c)
    return _PROGRAM_CACHE[key]


def run_device(image, t_sorted, M, b, src, dst, trace=False):
    (t_sorted, c1x, c2x, c1y, c2y, L, pimg, ph, pw, shift_r, shift_c,
     n_ray, n_int, n_col) = _prep(image, t_sorted, M, b, src, dst)

    assert n_ray % (N_CORES * P) == 0, n_ray
    rays_per_core = n_ray // N_CORES
    n_tiles = rays_per_core // P

    nc = _get_program(n_tiles, n_int, pw, pimg.shape[0], shift_r, shift_c)

    iota64 = np.arange(GR, dtype=np.float32).reshape(1, GR)
    in_maps = []
    for i in range(N_CORES):
        s = slice(i * rays_per_core, (i + 1) * rays_per_core)

        def plane(v):
            return v[s].reshape(n_tiles, P).T  # [p, a]

        cf = np.stack([plane(c1x), plane(c2x), plane(c1y), plane(c2y),
                       plane(L)], axis=1).reshape(P, 5 * n_tiles)
        in_maps.append({
            "t": t_sorted[s],
            "cf": np.ascontiguousarray(cf),
            "pimg": pimg,
            "iota64": iota64,
        })

    res = bass_utils.run_bass_kernel_spmd(nc, in_maps,
                                          core_ids=list(range(N_CORES)),
                                          trace=trace)
    out = np.concatenate([res.results[i]["out"] for i in range(N_CORES)])
    return out, res


def kernel(image, t_sorted, M, b, src, dst):
    out, _ = run_device(image, t_sorted, M, b, src, dst, trace=False)
    return out



# revision 7
# speedup vs baseline: 2.0148x; 2.0148x over previous
"""CT projector 2D forward (nn_CTProjector2D) — Trainium2 Bass kernel.

Strategy (8 NeuronCores, data-parallel over rays; image replicated):
  - 16384 rays/core, processed as 128 tiles of 128 rays x 768 t-values
    (rays in partitions, t along the free dimension).
  - Per-segment pixel indices are computed with DVE ops that reproduce the
    reference's fp32 arithmetic bit-exactly (for the graded M=I, b=0 case):
    x_t = t*ddx + sx, rowf = 0.5*(x_s+x_{s+1}), round via the +1.5*2^23
    magic constant (RNE, same as jnp.round).
  - Out-of-bounds segments cost nothing: we gather from a zero-padded image
    with guard bands, so OOB pixels read 0 exactly like the reference's
    `where(valid, ...)` masking.
  - The gather: Trainium has no per-element gather engine; the fastest
    measured primitive is the SWDGE `dma_gather` (MoE token gather) at its
    256 B minimum granule.  Each segment fetches its 64-px aligned block of
    the padded image from DRAM (int16 block ids, 4 parallel SWDGE queues),
    and the pixel is then selected on DVE with an iota/is_equal mask +
    multiply + inner-axis reduce.  The ucode wants indices "wrapped in 16
    partitions and replicated"; that layout is built on-device via a DRAM
    round-trip with strided access patterns.
  - seg_len = (t_{s+1}-t_s)*|dst-src| (== the reference's sqrt form to
    ~1e-7), fused with the per-ray reduction in one
    scalar_tensor_tensor(accum_out=...) op.
"""

import os
import sys

for _p in ("/opt/trn_rl_repo", "/root/.axon_site/_ro/trn_rl_repo"):
    if os.path.isdir(_p) and _p not in sys.path:
        sys.path.insert(0, _p)

import numpy as np

import concourse.bacc as bacc
import concourse.mybir as mybir
import concourse.tile as tile
from concourse import bass, bass_utils, library_config

F32 = mybir.dt.float32
I16 = mybir.dt.int16
OP = mybir.AluOpType

N_CORES = 8
P = 128                 # rays per tile (partitions)
GR = 64                 # gather granule: 64 f32 = 256 B (dma_gather minimum)
CH = 64                 # segments per dma_gather chunk
NCHUNK = 12             # chunks per tile (12*64 = 768 slots, 767 real)
MAGIC = np.float32(1.5 * 2.0**23)

_PROGRAM_CACHE: dict = {}


PREP_TRIGGER = os.environ.get("KERNEL_PREP_TRIGGER", "0") == "1"


def _build_program(n_tiles: int, n_int: int, pw: int, nblk: int,
                   shift_r: float, shift_c: float, n_queues: int = 4):
    nseg = n_int - 1          # 767
    nsegp = NCHUNK * CH       # 768 (padded segment slots)
    assert nsegp == n_int
    nc = bacc.Bacc("TRN2", target_bir_lowering=False, debug=False,
                   num_devices=N_CORES, num_swdge_queues=n_queues)

    t_d = nc.dram_tensor("t", [n_tiles * P, n_int], F32, kind="ExternalInput").ap()
    cf_d = nc.dram_tensor("cf", [P, 5 * n_tiles], F32, kind="ExternalInput").ap()
    img_d = nc.dram_tensor("pimg", [nblk, GR], F32, kind="ExternalInput").ap()
    io_d = nc.dram_tensor("iota64", [1, GR], F32, kind="ExternalInput").ap()
    out_d = nc.dram_tensor("out", [n_tiles * P], F32, kind="ExternalOutput").ap()

    t_v = t_d.rearrange("(a p) k -> a p k", p=P)
    out_v = out_d.rearrange("(a p) -> p a", p=P)

    with tile.TileContext(nc) as tc:
        with (
            tc.tile_pool(name="singles", bufs=1) as singles,
            tc.tile_pool(name="work", bufs=2) as work,
            tc.tile_pool(name="big", bufs=4) as big,
            tc.tile_pool(name="big2", bufs=2) as big2,
            tc.tile_pool(name="dram", bufs=2, space="DRAM") as dram,
        ):
            nc.gpsimd.load_library(library_config.mlp)
            dg_sems = [nc.alloc_semaphore(f"dg_dma_{q}") for q in range(n_queues)]
            cf = singles.tile([P, 5 * n_tiles], F32)
            nc.sync.dma_start(out=cf[:], in_=cf_d[:])
            sino = singles.tile([P, n_tiles], F32)
            iota64 = singles.tile([P, GR], F32)
            io_b = bass.AP(io_d.tensor, io_d.offset,
                           [[0, P], io_d.ap[1]])
            nc.sync.dma_start(out=iota64[:], in_=io_b)

            def cfs(j, a):
                k = j * n_tiles + a
                return cf[:, k:k + 1]

            inv64 = float(np.float32(1.0 / 64.0))
            for a in range(n_tiles):
                t_t = work.tile([P, n_int], F32, tag="t")
                nc.sync.dma_start(out=t_t[:], in_=t_v[a, :, :])

                # projected line coords (bit-exact with ref for M=I, b=0)
                x = work.tile([P, n_int], F32, tag="x")
                nc.vector.tensor_scalar(out=x[:], in0=t_t[:],
                                        scalar1=cfs(0, a), scalar2=cfs(1, a),
                                        op0=OP.mult, op1=OP.add)
                y = work.tile([P, n_int], F32, tag="y")
                nc.vector.tensor_scalar(out=y[:], in0=t_t[:],
                                        scalar1=cfs(2, a), scalar2=cfs(3, a),
                                        op0=OP.mult, op1=OP.add)
                # xs := row' = round(0.5*(x_s+x_{s+1})) + shift_r   (in-place)
                xs = work.tile([P, nsegp], F32, tag="xs")
                nc.vector.tensor_tensor(out=xs[:, 0:nseg], in0=x[:, 0:nseg],
                                        in1=x[:, 1:n_int], op=OP.add)
                nc.vector.memset(xs[:, nseg:nsegp], 0.0)
                nc.vector.tensor_scalar(out=xs[:], in0=xs[:],
                                        scalar1=0.5, scalar2=float(MAGIC),
                                        op0=OP.mult, op1=OP.add)
                nc.vector.tensor_scalar(out=xs[:], in0=xs[:],
                                        scalar1=float(MAGIC - np.float32(shift_r)),
                                        scalar2=None, op0=OP.subtract)
                ys = work.tile([P, nsegp], F32, tag="ys")
                nc.vector.tensor_tensor(out=ys[:, 0:nseg], in0=y[:, 0:nseg],
                                        in1=y[:, 1:n_int], op=OP.add)
                nc.vector.memset(ys[:, nseg:nsegp], 0.0)
                nc.vector.tensor_scalar(out=ys[:], in0=ys[:],
                                        scalar1=0.5, scalar2=float(MAGIC),
                                        op0=OP.mult, op1=OP.add)
                nc.vector.tensor_scalar(out=ys[:], in0=ys[:],
                                        scalar1=float(MAGIC - np.float32(shift_c)),
                                        scalar2=None, op0=OP.subtract)

                # flat = row'*PW + col'  (exact integer in f32)
                flat = work.tile([P, nsegp], F32, tag="flat")
                nc.vector.scalar_tensor_tensor(out=flat[:], in0=xs[:],
                                               scalar=float(pw), in1=ys[:],
                                               op0=OP.mult, op1=OP.add)
                # xs := blk = floor(flat/64)  (in-place reuse of xs)
                nc.vector.tensor_scalar(out=xs[:], in0=flat[:],
                                        scalar1=inv64, scalar2=-0.4921875,
                                        op0=OP.mult, op1=OP.add)
                nc.vector.tensor_scalar(out=xs[:], in0=xs[:],
                                        scalar1=float(MAGIC), scalar2=float(MAGIC),
                                        op0=OP.add, op1=OP.subtract)
                # m = flat - 64*blk
                m = work.tile([P, nsegp], F32, tag="m")
                nc.vector.scalar_tensor_tensor(out=m[:], in0=xs[:],
                                               scalar=-64.0, in1=flat[:],
                                               op0=OP.mult, op1=OP.add)
                blk16 = work.tile([P, nsegp], I16, tag="blk16")
                nc.vector.tensor_copy(blk16[:], xs[:])

                # dt
                dt = work.tile([P, nseg], F32, tag="dt")
                nc.vector.tensor_tensor(out=dt[:], in0=t_t[:, 1:n_int],
                                        in1=t_t[:, 0:nseg], op=OP.subtract)

                # ---- wrapped-index layout via on-chip partition fold ----
                # wrapped[q, g, c*8+k] = blk16[16k+q, CH*g+c]; the SWDGE ucode
                # reads its 16-partition group's copy, so the 16-partition
                # stream is then replicated to all 8 groups.
                # compute engines need 32-aligned partition bases; a plain
                # contiguous SBUF->SBUF DMA provides the odd-16 shift.
                blkB = work.tile([P, nsegp], I16, tag="blkB")
                nc.sync.dma_start(out=blkB[0:112, :], in_=blk16[16:128, :])
                wrapped = work.tile([P, NCHUNK, CH * 8], I16, tag="wrapped")
                wv = wrapped[:].rearrange("p g (c k) -> p g c k", k=8)
                for k in range(8):
                    src_t = blk16 if k % 2 == 0 else blkB
                    base = 16 * k if k % 2 == 0 else 16 * (k - 1)
                    nc.vector.tensor_copy(
                        wv[0:16, :, :, k],
                        src_t[base:base + 16, :].rearrange(
                            "q (g c) -> q g c", g=NCHUNK))
                for rep in range(1, 8):
                    nc.sync.dma_start(
                        out=wrapped[16 * rep:16 * rep + 16, :, :],
                        in_=wrapped[0:16, :, :])

                # ---- gather + select per chunk ----
                val = work.tile([P, nsegp], F32, tag="val")
                for g in range(NCHUNK):
                    g64 = big.tile([P, CH, GR], F32, tag="g64")
                    q = (a * NCHUNK + g) % n_queues
                    if PREP_TRIGGER:
                        nc.gpsimd.dma_gather(
                            out_ap=g64[:], in_ap=img_d[:, :],
                            idxs_ap=wrapped[:, g, :],
                            num_idxs=CH * P, num_idxs_reg=CH * P,
                            elem_size=GR, queue_num=q,
                            single_packet=False,
                            prepare_only=True, sem=dg_sems[q])
                        nc.gpsimd.trigger_dma(count=None, queue_num=q)
                    else:
                        nc.gpsimd.dma_gather(
                            out_ap=g64[:], in_ap=img_d[:, :],
                            idxs_ap=wrapped[:, g, :],
                            num_idxs=CH * P, num_idxs_reg=CH * P,
                            elem_size=GR, queue_num=q,
                            single_packet=False)
                    mask = big2.tile([P, CH, GR], F32, tag="mask")
                    i_ap = iota64[:]
                    iota_b = bass.AP(i_ap.tensor, i_ap.offset,
                                     [i_ap.ap[0], [0, CH], i_ap.ap[1]])
                    m_sl = m[:, g * CH:(g + 1) * CH]
                    m_b = bass.AP(m_sl.tensor, m_sl.offset,
                                  [m_sl.ap[0], m_sl.ap[1], [0, GR]])
                    nc.vector.tensor_tensor(out=mask[:], in0=iota_b, in1=m_b,
                                            op=OP.is_equal)
                    nc.vector.tensor_tensor(out=mask[:], in0=mask[:], in1=g64[:],
                                            op=OP.mult)
                    nc.vector.tensor_reduce(
                        out=val[:, g * CH:(g + 1) * CH],
                        in_=mask[:], op=OP.add, axis=mybir.AxisListType.X)

                # sino[:, a] = sum((val * L) * dt)
                scr = work.tile([P, nseg], F32, tag="scr")
                nc.vector.scalar_tensor_tensor(out=scr[:], in0=val[:, 0:nseg],
                                               scalar=cfs(4, a), in1=dt[:],
                                               op0=OP.mult, op1=OP.mult,
                                               accum_out=sino[:, a:a + 1])

            nc.sync.dma_start(out=out_v[:, :], in_=sino[:])

    nc.compile()
    return nc


def _prep(image, t_sorted, M, b, src, dst):
    """Host-side O(n_ray) prep: per-ray line coefficients + padded image."""
    f32 = np.float32
    image = np.ascontiguousarray(image, dtype=f32)
    t_sorted = np.ascontiguousarray(t_sorted, dtype=f32)
    M = np.asarray(M, dtype=f32)
    b = np.asarray(b, dtype=f32)
    src = np.asarray(src, dtype=f32)
    dst = np.asarray(dst, dtype=f32)

    n_row, n_col = image.shape
    n_ray, n_int = t_sorted.shape

    Minv = np.linalg.inv(M.astype(np.float64)).astype(f32)
    sx, sy = src[:, 0], src[:, 1]
    ddx = dst[:, 0] - sx
    ddy = dst[:, 1] - sy

    c1x = Minv[0, 0] * ddx + Minv[0, 1] * ddy
    c2x = (Minv[0, 0] * sx + Minv[0, 1] * sy) - (Minv[0, 0] * b[0] + Minv[0, 1] * b[1])
    c1y = Minv[1, 0] * ddx + Minv[1, 1] * ddy
    c2y = (Minv[1, 0] * sx + Minv[1, 1] * sy) - (Minv[1, 0] * b[0] + Minv[1, 1] * b[1])
    L = np.sqrt(ddx.astype(np.float64) ** 2 + ddy.astype(np.float64) ** 2).astype(f32)

    rlo = np.floor(min(np.min(c2x), np.min(c1x + c2x))) - 2.0
    rhi = np.ceil(max(np.max(c2x), np.max(c1x + c2x))) + 2.0
    clo = np.floor(min(np.min(c2y), np.min(c1y + c2y))) - 2.0
    chi = np.ceil(max(np.max(c2y), np.max(c1y + c2y))) + 2.0
    rlo, clo = min(rlo, 0.0), min(clo, 0.0)
    rhi, chi = max(rhi, float(n_row)), max(chi, float(n_col))
    shift_r = -rlo
    shift_c = -clo
    ph = int(rhi - rlo) + 2
    pw = int(chi - clo) + 2
    ph = -(-ph // 64) * 64
    pw = -(-pw // 64) * 64
    # block table must be int16-indexable
    assert ph * pw // GR < 32768, (ph, pw)

    pimg = np.zeros((ph, pw), dtype=f32)
    pimg[int(shift_r):int(shift_r) + n_row,
         int(shift_c):int(shift_c) + n_col] = image
    pimg = pimg.reshape(-1, GR)

    return (t_sorted, c1x, c2x, c1y, c2y, L, pimg, ph, pw, shift_r, shift_c,
            n_ray, n_int, n_col)


def _get_program(n_tiles, n_int, pw, nblk, shift_r, shift_c):
    key = (n_tiles, n_int, pw, nblk, float(shift_r), float(shift_c))
    if key not in _PROGRAM_CACHE:
        _PROGRAM_CACHE[key] = _build_program(n_tiles, n_int, pw, nblk,
                                             shift_r, shift_c)
    return _PROGRAM_CACHE[key]


def run_device(image, t_sorted, M, b, src, dst, trace=False):
    (t_sorted, c1x, c2x, c1y, c2y, L, pimg, ph, pw, shift_r, shift_c,
     n_ray, n_int, n_col) = _prep(image, t_sorted, M, b, src, dst)

    assert n_ray % (N_CORES * P) == 0, n_ray
    rays_per_core = n_ray // N_CORES
    n_tiles = rays_per_core // P

    nc = _get_program(n_tiles, n_int, pw, pimg.shape[0], shift_r, shift_c)

    iota64 = np.arange(GR, dtype=np.float32).reshape(1, GR)
    in_maps = []
    for i in range(N_CORES):
        s = slice(i * rays_per_core, (i + 1) * rays_per_core)

        def plane(v):
            return v[s].reshape(n_tiles, P).T  # [p, a]

        cf = np.stack([plane(c1x), plane(c2x), plane(c1y), plane(c2y),
                       plane(L)], axis=1).reshape(P, 5 * n_tiles)
        in_maps.append({
            "t": t_sorted[s],
            "cf": np.ascontiguousarray(cf),
            "pimg": pimg,
            "iota64": iota64,
        })

    res = bass_utils.run_bass_kernel_spmd(nc, in_maps,
                                          core_ids=list(range(N_CORES)),
                                          trace=trace)
    out = np.concatenate([res.results[i]["out"] for i in range(N_CORES)])
    return out, res


def kernel(image, t_sorted, M, b, src, dst):
    out, _ = run_device(image, t_sorted, M, b, src, dst, trace=False)
    return out

